# revision 8
# baseline (speedup 1.0000x reference)
"""Trainium2 Bass kernel: pairwise L2 distance (vq codebook lookup distances).

Computes dist[n, k] = || x[n, :] - centroids[k, :] ||_2 for
x: [8192, 512] f32, centroids: [128, 512] f32 -> dist: [8192, 128] f32.

Strategy (data parallel over 8 NeuronCores): shard x along N (1024 rows per
core), replicate centroids. Per core:
    dist^2[n,k] = |x_n|^2 + |c_k|^2 - 2 x_n . c_k
 - |x|^2 via ScalarE Square activation with accum_out (fused row-sum)
 - x . c^T via TensorE: transpose 128x128 x-tiles with PE (identity matmul),
   then 4 accumulating matmuls against pre-transposed (-2*c^T); a 5th rank-1
   matmul adds |c_k|^2 broadcast along partitions.
 - sqrt + |x_n|^2 bias fused in one ScalarE activation reading PSUM.
"""

import numpy as np

N, K, D = 8192, 512 // 4, 512  # K=128
NCORES = 8
NSHARD = N // NCORES  # 1024 rows per core
P = 128  # partitions / tile rows
NCHUNK = NSHARD // P  # 8 chunks of 128 rows per core
ND = D // P  # 4 contraction sub-tiles

_cache = {}


def _build_bass():
    from contextlib import ExitStack

    import concourse.mybir as mybir
    import concourse.tile as tile
    from concourse import bacc
    from concourse.masks import make_identity

    fp32 = mybir.dt.float32
    AF = mybir.ActivationFunctionType

    nc = bacc.Bacc(
        "TRN2",
        target_bir_lowering=False,
        debug=False,
        enable_asserts=False,
        num_devices=NCORES,
    )
    x_d = nc.dram_tensor("x", [NSHARD, D], fp32, kind="ExternalInput").ap()
    c_d = nc.dram_tensor("centroids", [K, D], fp32, kind="ExternalInput").ap()
    o_d = nc.dram_tensor("dist", [NSHARD, K], fp32, kind="ExternalOutput").ap()

    with tile.TileContext(nc) as tc, ExitStack() as ctx:
        singles = ctx.enter_context(tc.tile_pool(name="singles", bufs=1))
        xin = ctx.enter_context(tc.tile_pool(name="xin", bufs=4))
        sqp = ctx.enter_context(tc.tile_pool(name="sqp", bufs=3))
        xtp = ctx.enter_context(tc.tile_pool(name="xtp", bufs=4))
        xsqp = ctx.enter_context(tc.tile_pool(name="xsqp", bufs=4))
        doutp = ctx.enter_context(tc.tile_pool(name="doutp", bufs=3))
        ptp = ctx.enter_context(tc.tile_pool(name="ptp", bufs=3, space="PSUM"))
        poutp = ctx.enter_context(tc.tile_pool(name="poutp", bufs=3, space="PSUM"))
        prowp = ctx.enter_context(tc.tile_pool(name="prowp", bufs=1, space="PSUM"))

        # ---- one-time setup ----
        identity = singles.tile([P, P], fp32)
        make_identity(nc, identity[:])

        c_sb = singles.tile([K, D], fp32)
        nc.sync.dma_start(out=c_sb[:], in_=c_d)

        # csq_col[k] = sum_d c[k,d]^2  (ScalarE Square + fused row-sum)
        csq_col = singles.tile([K, 1], fp32)
        c_sq_scr = sqp.tile([K, D], fp32, tag="sq")
        nc.scalar.activation(
            c_sq_scr[:], c_sb[:], AF.Square, accum_out=csq_col[:]
        )

        # cT tiles, pre-scaled by -2:  m2cT[:, d, :] = -2 * c[:, d-block].T
        pt_c = ptp.tile([P, D], fp32, tag="pt")
        for d in range(ND):
            nc.tensor.transpose(
                pt_c[:, d * P : (d + 1) * P],
                c_sb[:, d * P : (d + 1) * P],
                identity[:],
            )
        m2cT = singles.tile([P, D], fp32)
        nc.scalar.mul(m2cT[:], pt_c[:], -2.0)

        # csq as a [1, K] row (PE transpose of the column) and a ones row.
        p_row = prowp.tile([1, K], fp32)
        nc.tensor.transpose(p_row[:], csq_col[:], identity[:])
        csq_row = singles.tile([1, K], fp32)
        nc.vector.tensor_copy(csq_row[:], p_row[:])
        ones_row = singles.tile([1, P], fp32)
        nc.vector.memset(ones_row[:], 1.0)

        # ---- main loop over 128-row chunks of this core's x shard ----
        # Software-pipelined: chunk i+1's PE transposes are emitted before
        # chunk i's matmuls so PE never stalls on the DVE psum->sbuf copy
        # (PE executes its stream in order; T(i+1) only needs DMA(i+1)).
        def load_and_transpose(i):
            rows = slice(i * P, (i + 1) * P)
            x_tile = xin.tile([P, D], fp32, tag="x")
            nc.sync.dma_start(out=x_tile[:, : D // 2], in_=x_d[rows, : D // 2])
            nc.sync.dma_start(out=x_tile[:, D // 2 :], in_=x_d[rows, D // 2 :])

            # xsq_col[n] = sum_d x[n,d]^2
            xsq_col = xsqp.tile([P, 1], fp32, tag="xsq")
            x_sq_scr = sqp.tile([P, D], fp32, tag="sq")
            nc.scalar.activation(
                x_sq_scr[:], x_tile[:], AF.Square, accum_out=xsq_col[:]
            )

            # transpose x chunk: 4x 128x128 PE transposes into one PSUM bank
            pt_x = ptp.tile([P, D], fp32, tag="pt")
            for d in range(ND):
                nc.tensor.transpose(
                    pt_x[:, d * P : (d + 1) * P],
                    x_tile[:, d * P : (d + 1) * P],
                    identity[:],
                )
            xT = xtp.tile([P, D], fp32, tag="xt")
            nc.vector.tensor_copy(xT[:], pt_x[:])
            return xT, xsq_col

        def matmul_and_store(i, xT, xsq_col):
            rows = slice(i * P, (i + 1) * P)
            # psum[n,k] = sum_d xT.T @ (-2 cT) + ones.T @ csq_row
            #          = -2 x.c + |c|^2
            pout = poutp.tile([P, K], fp32, tag="pout")
            for d in range(ND):
                nc.tensor.matmul(
                    pout[:],
                    xT[:, d * P : (d + 1) * P],
                    m2cT[:, d * P : (d + 1) * P],
                    start=(d == 0),
                    stop=False,
                )
            nc.tensor.matmul(
                pout[:], ones_row[:], csq_row[:], start=False, stop=True
            )

            # dist = sqrt(psum + xsq)   (bias = per-partition |x_n|^2)
            dist_sb = doutp.tile([P, K], fp32, tag="dist")
            nc.scalar.activation(
                dist_sb[:], pout[:], AF.Sqrt, bias=xsq_col[:], scale=1.0
            )
            nc.sync.dma_start(out=o_d[rows, :], in_=dist_sb[:])

        staged = load_and_transpose(0)
        for i in range(NCHUNK):
            nxt = load_and_transpose(i + 1) if i + 1 < NCHUNK else None
            matmul_and_store(i, *staged)
            staged = nxt

    nc.compile()
    return nc


def _get_nc():
    if "nc" not in _cache:
        _cache["nc"] = _build_bass()
    return _cache["nc"]


def kernel(**inputs) -> np.ndarray:
    from concourse.bass_utils import run_bass_kernel_spmd

    x = np.ascontiguousarray(inputs["x"], dtype=np.float32)
    c = np.ascontiguousarray(inputs["centroids"], dtype=np.float32)
    nc = _get_nc()
    in_maps = [
        {"x": np.ascontiguousarray(x[i * NSHARD : (i + 1) * NSHARD]), "centroids": c}
        for i in range(NCORES)
    ]
    res = run_bass_kernel_spmd(nc, in_maps, core_ids=list(range(NCORES)))
    return np.concatenate([r["dist"] for r in res.results], axis=0)
